# revision 21
# baseline (speedup 1.0000x reference)
"""Trainium2 Bass kernel for nn_CustomTripletLoss (B=16384, C=1000, D=1024).

Strategy (data-parallel over the anchor dim, 8 cores x 2048 anchors):
  For each anchor b:  d2[b, c] = |x_b|^2 - 2<x_b, t_c> + |t_c|^2
  The loss needs   d2_ap = d2[b, label_b]   and   d2_an = min_{c != label} d2[b, c].

  Per core the tensor engine computes P[b, c] = 2<x_b, t_c> (float32r matmuls,
  x transposed on-chip by the PE).  The DVE then forms, in one fused
  PSUM->SBUF move,   Qs[b, c] = P[b, c] - t2[c] * (c != label_b)
  using a GPSIMD-built mask, and vector.max gives the top-8 of each row:
    top0 = 2<x_b, t_label>          (t2 >= ~850 >> max |2S| makes it the max)
    top1 = max_{c != label} (2S - t2)
  so  d2_ap = x2 - top0 + t2[label]  and  d2_an = x2 - top1.
  The kernel exports top-8 rows, |x_b|^2, and t2; the host finishes with a
  t2[label] lookup plus sqrt/hinge/mean over the 16384 anchors (float64).
"""

import numpy as np

import concourse.bass as bass
import concourse.tile as tile
from concourse import bacc, mybir
from concourse.bass_utils import run_bass_kernel_spmd
from concourse.masks import make_identity

B, C, D = 16384, 1000, 1024
N_CORES = 8
BS = B // N_CORES          # 2048 anchors per core
NT = BS // 128             # 16 b-tiles per core
KT = D // 128              # 8 contraction chunks
CT = (C + 127) // 128      # 8 target row-tiles (last one 104 rows)
HALF = 500                 # free-dim half (one PSUM bank each, <=512)

F32 = mybir.dt.float32
F32R = mybir.dt.float32r


def build_program(repeat=1):
    """repeat>1 re-runs the main loop (same data, same outputs) so device-side
    per-pass time can be extracted by differencing two repeat counts."""
    nc = bacc.Bacc("TRN2", target_bir_lowering=False, debug=False)

    x_d = nc.dram_tensor("inputs", [BS, D], F32, kind="ExternalInput").ap()
    t_d = nc.dram_tensor("target", [C, D], F32, kind="ExternalInput").ap()
    lab_d = nc.dram_tensor("labels_f", [BS], F32, kind="ExternalInput").ap()
    omax_d = nc.dram_tensor("out_max8", [128, NT * 8], F32, kind="ExternalOutput").ap()
    ox2_d = nc.dram_tensor("out_x2", [128, NT], F32, kind="ExternalOutput").ap()
    ot2_d = nc.dram_tensor("out_t2", [C], F32, kind="ExternalOutput").ap()

    with tile.TileContext(nc) as tc:
        with (
            tc.tile_pool(name="consts", bufs=1) as consts,
            tc.tile_pool(name="tmat", bufs=1) as tmat,
            tc.tile_pool(name="sb", bufs=3) as sb,
            tc.tile_pool(name="outp", bufs=1) as outp,
            tc.tile_pool(name="dram", bufs=1, space="DRAM") as dram,
            tc.tile_pool(name="psum", bufs=2, space="PSUM") as psum,
        ):
            # ---- constants -------------------------------------------------
            ident = consts.tile([128, 128], F32)
            make_identity(nc, ident)

            iota_f = consts.tile([128, C], F32)
            nc.gpsimd.iota(
                iota_f,
                pattern=[[1, C]],
                base=0,
                channel_multiplier=0,
                allow_small_or_imprecise_dtypes=True,
            )

            lab_sb = consts.tile([128, NT], F32)
            nc.sync.dma_start(lab_sb, lab_d.rearrange("(i p) -> p i", p=128))
            # Copy labels onto DVE so downstream TensorScalarPtr ops don't
            # need DMA sync-waits (the TS ISA struct has too few wait slots).
            lab_v = consts.tile([128, NT], F32)
            nc.vector.tensor_copy(lab_v, lab_sb)

            # ---- target transpose + exact |t|^2 ---------------------------
            # tT[:, k, c] = target[c, 128k + dpart]  (fp32r, GEMM operand)
            # t2cols[p, j] = |target_{128j+p}|^2     (exact fp32)
            tT = tmat.tile([128, KT, C], F32R)
            t2cols = consts.tile([128, CT], F32)
            tsq = sb.tile([128, D], F32, tag="xsq")
            nc.vector.memset(t2cols, 0.0)
            for j in range(CT):
                cs = min(128, C - j * 128)
                t_str = sb.tile([128, D], F32, tag="tload")
                nc.sync.dma_start(t_str[:cs], t_d[j * 128 : j * 128 + cs, :])
                tsq = sb.tile([128, D], F32, tag="xsq")
                nc.scalar.activation(
                    tsq[:cs],
                    t_str[:cs],
                    mybir.ActivationFunctionType.Square,
                    accum_out=t2cols[:cs, j : j + 1],
                )
                for g in range(2):
                    pt = psum.tile([128, KT // 2, 128], F32, tag="xt", bufs=4)
                    for kk in range(KT // 2):
                        k = g * (KT // 2) + kk
                        nc.tensor.transpose(
                            pt[:, kk, :cs],
                            t_str[:cs, k * 128 : (k + 1) * 128],
                            ident[:cs, :cs],
                        )
                    nc.scalar.copy(
                        tT[:, g * (KT // 2) : (g + 1) * (KT // 2), j * 128 : j * 128 + cs],
                        pt[:, :, :cs],
                    )

            # Bounce t2 through DRAM to re-layout [c-part, tile] -> a
            # partition-broadcast row tile [128, C], and export it.
            t2_dram = dram.tile([CT * 128], F32)
            nc.sync.dma_start(
                t2_dram.rearrange("(t p) -> p t", p=128), t2cols
            )
            t2b = consts.tile([128, C], F32)
            nc.sync.dma_start(t2b, t2_dram[:C].unsqueeze(0).broadcast_to((128, C)))
            nc.sync.dma_start(ot2_d, t2_dram[:C])
            # negate once: mask wants -t2
            t2negb = consts.tile([128, C], F32)
            nc.vector.tensor_scalar_mul(t2negb, t2b, -1.0)

            # GPSIMD warmups: absorb the DVE/DMA waits so the per-tile mask
            # builds (TensorScalarPtr on Pool) need at most one sync wait.
            gw1 = consts.tile([128, 8], F32)
            nc.gpsimd.tensor_scalar(
                gw1, t2negb[:, :8], 0.0, None, mybir.AluOpType.add
            )
            gw2 = consts.tile([128, NT], F32)
            nc.gpsimd.tensor_scalar(gw2, lab_v, 0.0, None, mybir.AluOpType.add)

            # ---- outputs ---------------------------------------------------
            max8_sb = outp.tile([128, NT * 8], F32)
            x2cols = outp.tile([128, NT], F32)

            # ---- main loop over 16 b-tiles --------------------------------
            for ii in range(NT * repeat):
                i = ii % NT
                x_t = sb.tile([128, D], F32, tag="x")
                nc.sync.dma_start(x_t, x_d[i * 128 : (i + 1) * 128, :])

                # |x|^2 per anchor (ACT square + free-dim accumulate)
                xsq = sb.tile([128, D], F32, tag="xsq")
                nc.scalar.activation(
                    xsq,
                    x_t,
                    mybir.ActivationFunctionType.Square,
                    accum_out=x2cols[:, i : i + 1],
                )

                # mask: m[b, c] = -t2[c] where c != label_b, else 0
                # (compare on DVE — Pool lacks the scalar-ptr TS; multiply on
                #  the otherwise-idle GPSIMD)
                ne = sb.tile([128, C], F32, tag="ne")
                nc.vector.tensor_scalar(
                    ne, iota_f, lab_v[:, i : i + 1], None, mybir.AluOpType.not_equal
                )
                m_eq = sb.tile([128, C], F32, tag="m")
                nc.gpsimd.tensor_tensor(m_eq, ne, t2negb, mybir.AluOpType.mult)

                # transpose x tile: xt2[:, k, b] = 2 * x[b, 128k + dpart]
                # (two 1-bank PSUM groups so the ACT copy of group 0 overlaps
                #  the PE transposes of group 1)
                xt2 = sb.tile([128, KT, 128], F32R, tag="xt2")
                for g in range(2):
                    pxt = psum.tile([128, KT // 2, 128], F32, tag="xt", bufs=4)
                    for kk in range(KT // 2):
                        k = g * (KT // 2) + kk
                        nc.tensor.transpose(
                            pxt[:, kk, :], x_t[:, k * 128 : (k + 1) * 128], ident
                        )
                    nc.scalar.mul(
                        xt2[:, g * (KT // 2) : (g + 1) * (KT // 2)], pxt, 2.0
                    )

                # P = 2 x t^T  (accumulated in PSUM, two 500-wide banks)
                q_ps = psum.tile([128, 2, 512], F32, tag="q")
                for k in range(KT):
                    for h in range(2):
                        nc.tensor.matmul(
                            q_ps[:, h, :HALF],
                            lhsT=xt2[:, k, :],
                            rhs=tT[:, k, h * HALF : (h + 1) * HALF],
                            start=(k == 0),
                            stop=(k == KT - 1),
                        )

                # Qs = m + P   (PSUM -> SBUF move with mask folded in)
                qs = sb.tile([128, C], F32, tag="qs")
                for h in range(2):
                    nc.vector.scalar_tensor_tensor(
                        qs[:, h * HALF : (h + 1) * HALF],
                        m_eq[:, h * HALF : (h + 1) * HALF],
                        1.0,
                        q_ps[:, h, :HALF],
                        mybir.AluOpType.mult,
                        mybir.AluOpType.add,
                    )

                # top-8 of each row
                nc.vector.max(max8_sb[:, i * 8 : (i + 1) * 8], qs)

            nc.sync.dma_start(omax_d, max8_sb)
            nc.sync.dma_start(ox2_d, x2cols)

    nc.compile()
    return nc


_NC_CACHE = None


def _get_nc():
    global _NC_CACHE
    if _NC_CACHE is None:
        _NC_CACHE = build_program()
    return _NC_CACHE


def _postprocess(results, labels):
    lab = np.asarray(labels).astype(np.int64)
    total = 0.0
    for c in range(N_CORES):
        m8 = np.asarray(results[c]["out_max8"], dtype=np.float64).reshape(128, NT, 8)
        x2 = np.asarray(results[c]["out_x2"], dtype=np.float64)  # [128, NT]
        t2 = np.asarray(results[c]["out_t2"], dtype=np.float64)  # [C]
        top0 = m8[..., 0]
        top1 = m8[..., 1]
        # anchor b = core*BS + i*128 + p  ->  [p, i] layout
        lab_c = lab[c * BS : (c + 1) * BS].reshape(NT, 128).T  # [128, NT]
        d2_ap = np.maximum(x2 - top0 + t2[lab_c], 0.0)
        d2_an = np.maximum(x2 - top1, 0.0)
        per = np.maximum(np.sqrt(d2_ap) - np.sqrt(d2_an) + 1.0, 0.0)
        total += per.sum()
    return np.float32(total / B)


def run(inputs, labels, target, trace=False):
    nc = _get_nc()
    x = np.ascontiguousarray(np.asarray(inputs, dtype=np.float32))
    t = np.ascontiguousarray(np.asarray(target, dtype=np.float32))
    lab = np.ascontiguousarray(np.asarray(labels).astype(np.float32))
    assert x.shape == (B, D) and t.shape == (C, D) and lab.shape == (B,)

    in_maps = [
        {
            "inputs": x[c * BS : (c + 1) * BS],
            "labels_f": lab[c * BS : (c + 1) * BS],
            "target": t,
        }
        for c in range(N_CORES)
    ]
    res = run_bass_kernel_spmd(nc, in_maps, list(range(N_CORES)), trace=trace)
    return _postprocess(res.results, labels), res


def kernel(inputs, labels, target):
    out, _ = run(inputs, labels, target)
    return out


# revision 37
# speedup vs baseline: 1.2124x; 1.2124x over previous
"""Trainium2 Bass kernel for nn_CustomTripletLoss (B=16384, C=1000, D=1024).

Strategy (data-parallel over the anchor dim, 8 cores x 2048 anchors):
  For each anchor b:  d2[b, c] = |x_b|^2 - 2<x_b, t_c> + |t_c|^2
  The loss needs   d2_ap = d2[b, label_b]   and   d2_an = min_{c != label} d2[b, c].

  Per core the tensor engine computes P[b, c] = 2<x_b, t_c> (bf16 matmuls
  accumulated in fp32 PSUM, x transposed on-chip by the PE).  The DVE then
  forms, in one fused PSUM->SBUF move,
      Qs[b, c] = P[b, c] - t2[c] * (c != label_b)
  with a DVE-compare + GPSIMD-multiply mask (t2 itself stays exact fp32),
  and vector.max gives the top-8 of each row:
    top0 = 2<x_b, t_label>          (t2 >= ~850 >> max |2S| makes it the max)
    top1 = max_{c != label} (2S - t2)
  so  d2_ap = x2 - top0 + t2[label]  and  d2_an = x2 - top1.
  The kernel exports top-8 rows, |x_b|^2, and t2; the host finishes with a
  t2[label] lookup plus sqrt/hinge/mean over the 16384 anchors (float64).
"""

import numpy as np

import concourse.bass as bass
import concourse.tile as tile
from concourse import bacc, mybir
from concourse.bass_utils import run_bass_kernel_spmd
from concourse.masks import make_identity

B, C, D = 16384, 1000, 1024
N_CORES = 8
BS = B // N_CORES          # 2048 anchors per core
NT = BS // 128             # 16 b-tiles per core
KT = D // 128              # 8 contraction chunks
CT = (C + 127) // 128      # 8 target row-tiles (last one 104 rows)
HALF = 500                 # free-dim half (one PSUM bank each, <=512)

F32 = mybir.dt.float32
F32R = mybir.dt.float32r
BF16 = mybir.dt.bfloat16


def build_program(repeat=1, variant="full"):
    """repeat>1 re-runs the main loop (same data, same outputs) so device-side
    per-pass time can be extracted by differencing two repeat counts.
    variant: 'full' | 'notrans' | 'fewmm' | 'nodvetail' — timing ablations."""
    nc = bacc.Bacc("TRN2", target_bir_lowering=False, debug=False)

    x_d = nc.dram_tensor("inputs", [BS, D], F32, kind="ExternalInput").ap()
    t_d = nc.dram_tensor("target", [C, D], F32, kind="ExternalInput").ap()
    lab_d = nc.dram_tensor("labels_f", [BS], F32, kind="ExternalInput").ap()
    omax_d = nc.dram_tensor("out_max8", [128, NT * 8], F32, kind="ExternalOutput").ap()
    ox2_d = nc.dram_tensor("out_x2", [128, NT], F32, kind="ExternalOutput").ap()
    ot2_d = nc.dram_tensor("out_t2", [C], F32, kind="ExternalOutput").ap()

    with tile.TileContext(nc) as tc:
        with (
            tc.tile_pool(name="consts", bufs=1) as consts,
            tc.tile_pool(name="tmat", bufs=1) as tmat,
            tc.tile_pool(name="sb", bufs=3) as sb,
            tc.tile_pool(name="outp", bufs=1) as outp,
            tc.tile_pool(name="dram", bufs=1, space="DRAM") as dram,
            tc.tile_pool(name="psum", bufs=2, space="PSUM") as psum,
        ):
            # ---- constants -------------------------------------------------
            ident = consts.tile([128, 128], F32)
            make_identity(nc, ident)

            iota_f = consts.tile([128, C], F32)
            nc.gpsimd.iota(
                iota_f,
                pattern=[[1, C]],
                base=0,
                channel_multiplier=0,
                allow_small_or_imprecise_dtypes=True,
            )

            lab_sb = consts.tile([128, NT], F32)
            nc.sync.dma_start(lab_sb, lab_d.rearrange("(i p) -> p i", p=128))
            # Copy labels onto DVE so downstream TensorScalarPtr ops don't
            # need DMA sync-waits (the TS ISA struct has too few wait slots).
            lab_v = consts.tile([128, NT], F32)
            nc.vector.tensor_copy(lab_v, lab_sb)

            # ---- target transpose + exact |t|^2 ---------------------------
            # tT[:, k, c] = target[c, 128k + dpart]  (fp32r, GEMM operand)
            # t2cols[p, j] = |target_{128j+p}|^2     (exact fp32)
            tT = tmat.tile([128, KT, C], BF16)
            t2cols = consts.tile([128, CT], F32)
            nc.vector.memset(t2cols, 0.0)
            for j in range(CT):
                cs = min(128, C - j * 128)
                t_str = sb.tile([128, D], F32, tag="tload")
                nc.sync.dma_start(t_str[:cs], t_d[j * 128 : j * 128 + cs, :])
                tsq = sb.tile([128, D], F32, tag="xsq")
                nc.scalar.activation(
                    tsq[:cs],
                    t_str[:cs],
                    mybir.ActivationFunctionType.Square,
                    accum_out=t2cols[:cs, j : j + 1],
                )
                for g in range(2):
                    pt = psum.tile(
                        [128, KT // 2, 128], F32, tag="xt",
                        bufs=(2 if variant == "fullq3" else 4),
                    )
                    for kk in range(KT // 2):
                        k = g * (KT // 2) + kk
                        nc.tensor.transpose(
                            pt[:, kk, :cs],
                            t_str[:cs, k * 128 : (k + 1) * 128],
                            ident[:cs, :cs],
                        )
                    nc.scalar.copy(
                        tT[:, g * (KT // 2) : (g + 1) * (KT // 2), j * 128 : j * 128 + cs],
                        pt[:, :, :cs],
                    )

            # Bounce t2 through DRAM to re-layout [c-part, tile] -> a
            # partition-broadcast row tile [128, C], and export it.
            t2_dram = dram.tile([CT * 128], F32)
            nc.sync.dma_start(
                t2_dram.rearrange("(t p) -> p t", p=128), t2cols
            )
            t2b = consts.tile([128, C], F32)
            nc.sync.dma_start(t2b, t2_dram[:C].unsqueeze(0).broadcast_to((128, C)))
            nc.sync.dma_start(ot2_d, t2_dram[:C])
            # negate once: mask wants -t2
            t2negb = consts.tile([128, C], F32)
            nc.vector.tensor_scalar_mul(t2negb, t2b, -1.0)

            # GPSIMD warmups: absorb the DVE/DMA waits so the per-tile mask
            # builds (TensorScalarPtr on Pool) need at most one sync wait.
            gw1 = consts.tile([128, 8], F32)
            nc.gpsimd.tensor_scalar(
                gw1, t2negb[:, :8], 0.0, None, mybir.AluOpType.add
            )
            gw2 = consts.tile([128, NT], F32)
            nc.gpsimd.tensor_scalar(gw2, lab_v, 0.0, None, mybir.AluOpType.add)

            # ---- outputs ---------------------------------------------------
            max8_sb = outp.tile([128, NT * 8], F32)
            x2cols = outp.tile([128, NT], F32)
            if variant in ("nodvetail", "mmonly", "dmaonly"):
                nc.vector.memset(max8_sb, 0.0)
            if variant in ("mmonly", "dmaonly"):
                nc.vector.memset(x2cols, 0.0)

            # ---- main loop over 16 b-tiles, software-pipelined -------------
            # Stage A (tile i): DMA load, |x|^2, PE transposes, ACT copy.
            # Stage B (tile i-1): GEMM, mask+move, top-8.  Emitting A(i)
            # before B(i-1) lets the PE run transposes of tile i while tile
            # i-1's xt2 copy finishes, so the PE never stalls on ACT.
            n_iter = NT * repeat
            xt2_prev = None

            xt2_const = None
            if variant in ("notrans", "mmonly", "dmaonly"):
                xt2_const = tmat.tile([128, KT, 128], BF16)
                nc.vector.memset(xt2_const, 0.5)

            def stage_a(i):
                x_t = sb.tile([128, D], F32, tag="x")
                nc.sync.dma_start(x_t, x_d[i * 128 : (i + 1) * 128, :])
                if variant in ("mmonly", "dmaonly"):
                    return xt2_const

                # |x|^2 per anchor (ACT square + free-dim accumulate)
                xsq = sb.tile([128, D], F32, tag="xsq")
                nc.scalar.activation(
                    xsq,
                    x_t,
                    mybir.ActivationFunctionType.Square,
                    accum_out=x2cols[:, i : i + 1],
                )
                if variant == "notrans":
                    return xt2_const

                # transpose x tile: xt2[:, k, b] = 2 * x[b, 128k + dpart]
                xt2 = sb.tile([128, KT, 128], BF16, tag="xt2")
                for g in range(2):
                    pxt = psum.tile(
                        [128, KT // 2, 128], F32, tag="xt",
                        bufs=(2 if variant == "fullq3" else 4),
                    )
                    for kk in range(KT // 2):
                        k = g * (KT // 2) + kk
                        nc.tensor.transpose(
                            pxt[:, kk, :], x_t[:, k * 128 : (k + 1) * 128], ident
                        )
                    nc.scalar.mul(
                        xt2[:, g * (KT // 2) : (g + 1) * (KT // 2)], pxt, 2.0
                    )
                return xt2

            def stage_b(i, xt2):
                if variant == "dmaonly":
                    return
                n_k = 1 if variant == "fewmm" else KT
                # P = 2 x t^T  (accumulated in PSUM, two 500-wide banks)
                q_ps = psum.tile(
                    [128, 2, 512], F32, tag="q",
                    bufs=(3 if variant == "fullq3" else None),
                )
                if variant == "mmswap":
                    for h in range(2):
                        for k in range(n_k):
                            nc.tensor.matmul(
                                q_ps[:, h, :HALF],
                                lhsT=xt2[:, k, :],
                                rhs=tT[:, k, h * HALF : (h + 1) * HALF],
                                start=(k == 0),
                                stop=(k == n_k - 1),
                            )
                else:
                    for k in range(n_k):
                        for h in range(2):
                            nc.tensor.matmul(
                                q_ps[:, h, :HALF],
                                lhsT=xt2[:, k, :],
                                rhs=tT[:, k, h * HALF : (h + 1) * HALF],
                                start=(k == 0),
                                stop=(k == n_k - 1),
                            )
                if variant in ("nodvetail", "mmonly"):
                    return

                # mask: m[b, c] = -t2[c] where c != label_b, else 0
                # (compare on DVE — Pool lacks the scalar-ptr TS; multiply on
                #  the otherwise-idle GPSIMD)
                ne = sb.tile([128, C], F32, tag="ne")
                nc.vector.tensor_scalar(
                    ne, iota_f, lab_v[:, i : i + 1], None, mybir.AluOpType.not_equal
                )
                m_eq = sb.tile([128, C], F32, tag="m")
                nc.gpsimd.tensor_tensor(m_eq, ne, t2negb, mybir.AluOpType.mult)

                # Qs = m + P   (PSUM -> SBUF move with mask folded in)
                qs = sb.tile([128, C], F32, tag="qs")
                for h in range(2):
                    nc.vector.scalar_tensor_tensor(
                        qs[:, h * HALF : (h + 1) * HALF],
                        m_eq[:, h * HALF : (h + 1) * HALF],
                        1.0,
                        q_ps[:, h, :HALF],
                        mybir.AluOpType.mult,
                        mybir.AluOpType.add,
                    )

                # top-8 of each row
                nc.vector.max(max8_sb[:, i * 8 : (i + 1) * 8], qs)

            for ii in range(n_iter + 1):
                if ii < n_iter:
                    xt2_new = stage_a(ii % NT)
                else:
                    xt2_new = None
                if ii > 0:
                    stage_b((ii - 1) % NT, xt2_prev)
                xt2_prev = xt2_new

            nc.sync.dma_start(omax_d, max8_sb)
            nc.sync.dma_start(ox2_d, x2cols)

    nc.compile()
    return nc


_NC_CACHE = None


def _get_nc():
    global _NC_CACHE
    if _NC_CACHE is None:
        _NC_CACHE = build_program()
    return _NC_CACHE


def _postprocess(results, labels):
    lab = np.asarray(labels).astype(np.int64)
    total = 0.0
    for c in range(N_CORES):
        m8 = np.asarray(results[c]["out_max8"], dtype=np.float64).reshape(128, NT, 8)
        x2 = np.asarray(results[c]["out_x2"], dtype=np.float64)  # [128, NT]
        t2 = np.asarray(results[c]["out_t2"], dtype=np.float64)  # [C]
        top0 = m8[..., 0]
        top1 = m8[..., 1]
        # anchor b = core*BS + i*128 + p  ->  [p, i] layout
        lab_c = lab[c * BS : (c + 1) * BS].reshape(NT, 128).T  # [128, NT]
        d2_ap = np.maximum(x2 - top0 + t2[lab_c], 0.0)
        d2_an = np.maximum(x2 - top1, 0.0)
        per = np.maximum(np.sqrt(d2_ap) - np.sqrt(d2_an) + 1.0, 0.0)
        total += per.sum()
    return np.float32(total / B)


def run(inputs, labels, target, trace=False):
    nc = _get_nc()
    x = np.ascontiguousarray(np.asarray(inputs, dtype=np.float32))
    t = np.ascontiguousarray(np.asarray(target, dtype=np.float32))
    lab = np.ascontiguousarray(np.asarray(labels).astype(np.float32))
    assert x.shape == (B, D) and t.shape == (C, D) and lab.shape == (B,)

    in_maps = [
        {
            "inputs": x[c * BS : (c + 1) * BS],
            "labels_f": lab[c * BS : (c + 1) * BS],
            "target": t,
        }
        for c in range(N_CORES)
    ]
    res = run_bass_kernel_spmd(nc, in_maps, list(range(N_CORES)), trace=trace)
    return _postprocess(res.results, labels), res


def kernel(inputs, labels, target):
    out, _ = run(inputs, labels, target)
    return out
